# revision 24
# baseline (speedup 1.0000x reference)
"""Trainium2 Bass kernel for nn_DCMModle (dense_cnn, DCM dynamic-filter module).

Reference computation (B=8, XC=1024, YC=512, C=512, H=W=64, P=H*W=4096):
  gf  = relu(BN_gen(w_gen @ mean_hw(y) + b_gen))          per-sample [C]
  xr  = relu(BN_red(w_red @ x + b_red))                   [C, P]
  z   = relu(BN_act(xr * gf))                             [C, P]
  out = relu(BN_fus(w_fus @ z + b_fus))                   [C, P]

Strategy:
  - Data-parallel over batch: core b computes sample b. No collectives.
  - All BatchNorms folded into conv weights/biases on the host (pure affine).
  - Matmuls run in float32r (paired-bf16 fp32 mode: bf16 speed, ~2e-4 rel err).
  - Fully fused device pipeline: x streamed in 512-pixel chunks,
    red-conv -> scale/shift epilogues -> fus-conv -> store, all on-chip.
"""

import os
import sys
import time

for _p in ("/opt/trn_rl_repo", os.path.expanduser("~/.axon_site/_ro/trn_rl_repo")):
    if os.path.isdir(_p) and _p not in sys.path:
        sys.path.insert(0, _p)
        break

import ml_dtypes
import numpy as np

import concourse.bass as bass
import concourse.tile as tile
from concourse import bacc, mybir
from concourse.bass2jax import _bass_exec_p, install_neuronx_cc_hook, partition_id_tensor

F32 = mybir.dt.float32
F32R = mybir.dt.float32r
BF16 = mybir.dt.bfloat16
AF = mybir.ActivationFunctionType
ALU = mybir.AluOpType

B, XC, YC, C, H, W = 8, 1024, 512, 512, 64, 64
P = H * W          # 4096 pixels per sample
NCORES = 8
EPS = 1e-5

NKX = XC // 128    # 8 k-chunks for the reduce conv
NKC = C // 128     # 4 chunks of the C=512 channel dim
PCH = 512          # pixel chunk (one PSUM bank of fp32)
NP = P // PCH      # 8 pixel chunks


HALF = P // 2          # 2048 columns per y staging piece


def _build_nc(rep=1, timing=False):
    nc = bacc.Bacc("TRN2", target_bir_lowering=False, debug=False,
                   num_devices=NCORES)

    # timing builds keep the big tensors device-internal so per-call wall
    # time isn't dominated by shipping them through the axon tunnel
    big = "Internal" if timing else "ExternalInput"
    big_out = "Internal" if timing else "ExternalOutput"
    # x and the weights ship bf16: the matmuls already run at bf16-pair
    # precision (fp32r), so feeding bf16 operands costs ~3e-3 rel err while
    # halving the dominant HBM streams and dropping the fp32->fp32r rounding
    # copies from the vector engine.
    xb = nc.dram_tensor("xb", [XC, P], BF16, kind=big)
    yb = nc.dram_tensor("yb", [YC, P], BF16, kind=big)
    wrT = nc.dram_tensor("wrT", [XC, C], BF16, kind="ExternalInput")
    wgT = nc.dram_tensor("wgT", [YC, C], BF16, kind="ExternalInput")
    wfT = nc.dram_tensor("wfT", [C, C], BF16, kind="ExternalInput")
    # packed per-channel constants, [128, 5*NKC]:
    # cols [0:4) b_red', [4:8) b_gen', [8:12) a_act, [12:16) c_act, [16:20) b_fus'
    cst = nc.dram_tensor("cst", [128, 5 * NKC], F32, kind="ExternalInput")
    ob = nc.dram_tensor("ob", [C, P], F32, kind=big_out)
    dummy = None
    if timing:
        dummy = nc.dram_tensor("tout", [128, 128], F32, kind="ExternalOutput")

    x_v = xb.ap().rearrange("(k p) n -> p k n", p=128)    # [128, NKX, P]
    y_v = yb.ap().rearrange("(q p) n -> p q n", p=128)    # [128, NKC, P]
    wr_v = wrT.ap().rearrange("(k p) m -> p k m", p=128)  # [128, NKX, C]
    wg_v = wgT.ap().rearrange("(k p) m -> p k m", p=128)  # [128, NKC, C]
    wf_v = wfT.ap().rearrange("(k p) m -> p k m", p=128)  # [128, NKC, C]
    o_v = ob.ap().rearrange("(m p) n -> p m n", p=128)    # [128, NKC, P]

    with tile.TileContext(nc) as tc:
        with (
            tc.tile_pool(name="const", bufs=1) as constp,
            tc.tile_pool(name="stage", bufs=2) as stagep,   # y staging pieces
            tc.tile_pool(name="xin", bufs=3) as xinp,
            tc.tile_pool(name="xrel", bufs=28) as xrelp,
            tc.tile_pool(name="z", bufs=2) as zp,
            tc.tile_pool(name="out", bufs=2) as outp,
            tc.tile_pool(name="rps", bufs=4, space="PSUM") as rpsp,
            tc.tile_pool(name="fps", bufs=3, space="PSUM") as fpsp,
            tc.tile_pool(name="gps", bufs=1, space="PSUM") as gpsp,
        ):
            # ---- constants ----
            cs = constp.tile([128, 5 * NKC], F32)
            nc.sync.dma_start(cs[:], cst.ap())
            c_bred = lambda m: cs[:, m:m + 1]
            c_bgen = lambda m: cs[:, NKC + m:NKC + m + 1]
            c_aact = cs[:, 2 * NKC:3 * NKC]
            c_cact = lambda m: cs[:, 3 * NKC + m:3 * NKC + m + 1]
            c_bfus = lambda m: cs[:, 4 * NKC + m:4 * NKC + m + 1]

            # rep>1 wraps the whole body in a hardware loop (timing builds
            # only): per-pass time == one cold kernel execution.
            import contextlib
            loop_cm = tc.For_i(0, rep, 1) if rep > 1 else contextlib.nullcontext()
            loop_cm.__enter__()

            # ---- reduce-conv weights first on the x (SP) queue ----
            wr_r = constp.tile([128, NKX, C], BF16)
            nc.sync.dma_start(wr_r[:], wr_v)

            gft = constp.tile([128, NKC], F32)
            s_t = constp.tile([128, NKC], F32)

            # ---- phase B: main pixel-chunk pipeline, software-pipelined.
            # head(pi): x DMA -> reduce matmuls -> xq = relu(.+b) drains PSUM
            # immediately (no s_t dependency, so the PE never blocks on the
            # y path). tail(pi): zt = relu(s*xq+c) -> fusion conv -> store.
            # Emission order IS the DMA-pool priority: wr, x0, x1 go first so
            # the PE starts asap; the y/wg/wf stream fills the DMA slack while
            # the first two reduce-convs run; the gen conv then slots into the
            # PE queue before the tails.
            def emit_head(pi):
                px = slice(pi * PCH, (pi + 1) * PCH)
                xt = xinp.tile([128, NKX, PCH], BF16, tag="xt")
                nc.sync.dma_start(xt[:], x_v[:, :, px])
                xqs = []
                for m in range(NKC):
                    ps = rpsp.tile([128, PCH], F32)
                    for k in range(NKX):
                        nc.tensor.matmul(
                            ps[:], wr_r[:, k, m * 128:(m + 1) * 128],
                            xt[:, k, :], start=(k == 0), stop=(k == NKX - 1))
                    xq = xrelp.tile([128, PCH], F32)
                    nc.vector.tensor_scalar(xq[:], ps[:], c_bred(m), 0.0,
                                            op0=ALU.add, op1=ALU.max)
                    xqs.append(xq)
                return xqs

            def emit_tail(pi, xqs):
                px = slice(pi * PCH, (pi + 1) * PCH)
                zt = zp.tile([128, NKC, PCH], BF16)
                for m in range(NKC):
                    nc.scalar.activation(zt[:, m, :], xqs[m][:], AF.Relu,
                                         bias=c_cact(m), scale=s_t[:, m:m + 1])
                ot = outp.tile([128, NKC, PCH], F32)
                for m in range(NKC):
                    ps2 = fpsp.tile([128, PCH], F32)
                    for k in range(NKC):
                        nc.tensor.matmul(ps2[:], wf_r[:, k, m * 128:(m + 1) * 128],
                                         zt[:, k, :], start=(k == 0),
                                         stop=(k == NKC - 1))
                    nc.vector.tensor_scalar(ot[:, m, :], ps2[:], c_bfus(m), 0.0,
                                            op0=ALU.add, op1=ALU.max)
                nc.gpsimd.dma_start(o_v[:, :, px], ot[:])

            PRE = 2
            heads = [emit_head(pi) for pi in range(PRE)]

            # ---- phase A: y avg-pool + gen/fus weights on the Act queue ----
            ypp = constp.tile([128, NKC, 2], F32)
            for q in range(NKC):
                for h in range(2):
                    ystg = stagep.tile([128, HALF], BF16, tag="ystage")
                    nc.scalar.dma_start(
                        ystg[:], y_v[:, q, h * HALF:(h + 1) * HALF])
                    nc.vector.reduce_sum(ypp[:, q, h:h + 1], ystg[:],
                                         axis=mybir.AxisListType.X)
            ypool = constp.tile([128, NKC], F32)
            nc.vector.tensor_add(ypool[:], ypp[:, :, 0], ypp[:, :, 1])
            # rounded bf16 copy, N=2 moving columns
            ypr = constp.tile([128, NKC, 2], BF16)
            for j in range(2):
                nc.vector.tensor_copy(ypr[:, :, j], ypool[:])

            wg_r = constp.tile([128, NKC, C], BF16)
            nc.scalar.dma_start(wg_r[:], wg_v)
            wf_r = constp.tile([128, NKC, C], BF16)
            nc.scalar.dma_start(wf_r[:], wf_v)

            # gen conv: 16 tiny matmuls, all accumulating in one PSUM bank
            gp = gpsp.tile([128, NKC, 2], F32)
            for m in range(NKC):
                for q in range(NKC):
                    nc.tensor.matmul(
                        gp[:, m, :], wg_r[:, q, m * 128:(m + 1) * 128],
                        ypr[:, q, :], start=(q == 0), stop=(q == NKC - 1))
            for m in range(NKC):
                nc.scalar.activation(gft[:, m:m + 1], gp[:, m, 0:1],
                                     AF.Relu, bias=c_bgen(m))
            nc.vector.tensor_mul(s_t[:], gft[:], c_aact)

            for pi in range(PRE):
                emit_tail(pi, heads[pi])
            for pi in range(PRE, NP):
                emit_tail(pi, emit_head(pi))

            loop_cm.__exit__(None, None, None)

            if dummy is not None:
                dt_ = constp.tile([128, 128], F32)
                nc.vector.memset(dt_[:], 0.0)
                nc.gpsimd.dma_start(dummy.ap(), dt_[:])

    nc.compile()
    return nc


_CACHE = {}


def _get_runner(rep=1, timing=False):
    """Build (once) the jitted 8-core SPMD executable. Returns a callable
    taking concatenated-along-axis-0 per-core input arrays."""
    key = ("runner", rep, timing)
    if key in _CACHE:
        return _CACHE[key]

    import jax
    from jax.experimental.shard_map import shard_map
    from jax.sharding import Mesh, PartitionSpec

    install_neuronx_cc_hook()
    nc = _build_nc(rep=rep, timing=timing)

    part_name = nc.partition_id_tensor.name if nc.partition_id_tensor else None
    in_names, out_names, out_avals, zero_outs = [], [], [], []
    for alloc in nc.m.functions[0].allocations:
        if not isinstance(alloc, mybir.MemoryLocationSet):
            continue
        name = alloc.memorylocations[0].name
        if alloc.kind == "ExternalInput":
            if name != part_name:
                in_names.append(name)
        elif alloc.kind == "ExternalOutput":
            shape = tuple(alloc.tensor_shape)
            dtype = mybir.dt.np(alloc.dtype)
            out_names.append(name)
            out_avals.append(jax.core.ShapedArray(shape, dtype))
            zero_outs.append(np.zeros(shape, dtype))
    n_params = len(in_names)
    all_in_names = in_names + out_names
    if part_name is not None:
        all_in_names = all_in_names + [part_name]

    def _body(*args):
        operands = list(args)
        if part_name is not None:
            operands.append(partition_id_tensor())
        outs = _bass_exec_p.bind(
            *operands,
            out_avals=tuple(out_avals),
            in_names=tuple(all_in_names),
            out_names=tuple(out_names),
            lowering_input_output_aliases=(),
            sim_require_finite=True,
            sim_require_nnan=True,
            nc=nc,
        )
        return tuple(outs)

    devices = jax.devices()[:NCORES]
    mesh = Mesh(np.asarray(devices), ("core",))
    n_all = n_params + len(out_names)
    fn = jax.jit(
        shard_map(_body, mesh=mesh,
                  in_specs=(PartitionSpec("core"),) * n_all,
                  out_specs=(PartitionSpec("core"),) * len(out_names),
                  check_rep=False),
        keep_unused=True,
    )
    _CACHE[key] = (fn, in_names, out_names, zero_outs)
    return _CACHE[key]


def _prep_inputs(x, y, w_red, b_red, g_red, be_red, m_red, v_red,
                 w_gen, b_gen, g_gen, be_gen, m_gen, v_gen,
                 g_act, be_act, m_act, v_act,
                 w_fus, b_fus, g_fus, be_fus, m_fus, v_fus):
    """Fold BN into conv weights/biases; build per-core input dict."""
    f = np.float32

    def fold(w, b, g, be, m, v):
        a = (g / np.sqrt(v + EPS)).astype(f)
        wT = np.ascontiguousarray((a[:, None] * w).T.astype(f))
        bias = (a * (b - m) + be).astype(f)
        return wT, bias

    bf16 = ml_dtypes.bfloat16
    wrT, br = fold(w_red, b_red, g_red, be_red, m_red, v_red)
    wgT, bg = fold(w_gen, b_gen, g_gen, be_gen, m_gen, v_gen)
    wgT = (wgT / np.float32(P)).astype(f)      # fold the avg-pool 1/HW
    wfT, bf = fold(w_fus, b_fus, g_fus, be_fus, m_fus, v_fus)
    wrT = wrT.astype(bf16)
    wgT = wgT.astype(bf16)
    wfT = wfT.astype(bf16)
    a_act = (g_act / np.sqrt(v_act + EPS)).astype(f)
    c_act = (be_act - a_act * m_act).astype(f)

    def pack(v):  # [C] -> [128, NKC] (column m = channels m*128:(m+1)*128)
        return np.ascontiguousarray(v.reshape(NKC, 128).T)

    cstv = np.concatenate(
        [pack(br), pack(bg), pack(a_act), pack(c_act), pack(bf)], axis=1
    ).astype(f)

    shared = {"wrT": wrT, "wgT": wgT, "wfT": wfT, "cst": cstv}
    per_core = []
    for b_ in range(B):
        m_ = dict(shared)
        m_["xb"] = np.asarray(x[b_].reshape(XC, P), dtype=bf16)
        m_["yb"] = np.asarray(y[b_].reshape(YC, P), dtype=bf16)
        per_core.append(m_)
    return per_core


def _run(per_core_maps, iters=1, rep=1, timing=False):
    """Execute the SPMD program; returns (list of per-core output dicts,
    per-iteration wall seconds over `iters` chained dispatches)."""
    import jax
    from jax.sharding import Mesh, NamedSharding, PartitionSpec

    fn, in_names, out_names, zero_outs = _get_runner(rep=rep, timing=timing)
    concat_in = [
        np.concatenate([np.asarray(per_core_maps[c][n]) for c in range(NCORES)], axis=0)
        for n in in_names
    ]
    concat_zero = [
        np.zeros((NCORES * z.shape[0], *z.shape[1:]), z.dtype) for z in zero_outs
    ]
    # Shard explicitly: without a sharding, arrays commit to device 0 and
    # every call pays a full cross-core reshard through the tunnel (~25ms).
    mesh = Mesh(np.asarray(jax.devices()[:NCORES]), ("core",))
    sh = NamedSharding(mesh, PartitionSpec("core"))
    args = [jax.device_put(a, sh) for a in concat_in + concat_zero]
    out = fn(*args)
    jax.block_until_ready(out)
    dt = None
    if iters > 1:
        t0 = time.perf_counter()
        for _ in range(iters):
            out = fn(*args)
        jax.block_until_ready(out)
        dt = (time.perf_counter() - t0) / iters
    outs_np = [np.asarray(o) for o in out]
    results = [
        {n: outs_np[i].reshape(NCORES, -1, outs_np[i].shape[-1])[c]
         for i, n in enumerate(out_names)}
        for c in range(NCORES)
    ]
    return results, dt


def kernel(**inputs):
    per_core = _prep_inputs(**inputs)
    results, _ = _run(per_core)
    out = np.stack([results[c]["ob"].reshape(C, H, W) for c in range(B)])
    return out.astype(np.float32)


def kernel_timed(inputs, iters=1, rep=32768, warm=1, chains=3):
    """Measure steady-state HW execution time per full kernel pass.

    Builds the kernel with the whole pipeline wrapped in a `rep`-iteration
    hardware loop (every pass re-reads the real inputs from HBM and rewrites
    the real output), so one dispatch performs `rep` complete executions and
    the ~2ms per-dispatch tunnel/host overhead is amortized away. Returns
    (output of the same timed build, seconds per single execution).
    """
    import jax

    per_core = _prep_inputs(**inputs)
    results, _ = _run(per_core, rep=rep)
    out = np.stack([results[c]["ob"].reshape(C, H, W) for c in range(B)])

    fn, in_names, out_names, zero_outs = _get_runner(rep=rep, timing=False)
    from jax.sharding import Mesh, NamedSharding, PartitionSpec
    mesh = Mesh(np.asarray(jax.devices()[:NCORES]), ("core",))
    sh = NamedSharding(mesh, PartitionSpec("core"))
    concat_in = [
        np.concatenate([np.asarray(per_core[c][n]) for c in range(NCORES)], axis=0)
        for n in in_names
    ]
    concat_zero = [
        np.zeros((NCORES * z.shape[0], *z.shape[1:]), z.dtype) for z in zero_outs
    ]
    args = [jax.device_put(a, sh) for a in concat_in + concat_zero]
    jax.block_until_ready(args)
    for _ in range(warm):
        o = fn(*args)
    jax.block_until_ready(o)
    samples = []
    for _ in range(chains):
        t0 = time.perf_counter()
        for _ in range(iters):
            o = fn(*args)
        jax.block_until_ready(o)
        samples.append((time.perf_counter() - t0) / (iters * rep))
    samples.sort()
    dt = samples[len(samples) // 2]  # median chain
    return out.astype(np.float32), dt

